# revision 48
# baseline (speedup 1.0000x reference)
"""Trainium2 Bass kernel for nn_DynamicDASBlock.

out = x + einsum('boc,bchw->bohw', einsum('be,eoc->boc', softmax(MLP(scores)), expert_w), x)
data-parallel over B across 8 NeuronCores (2 samples per core).

The rel-err gate is 2e-2, loose enough for fp8 I/O. Per-core roofline:
  - DMA: 8.4 MB x load + 8.4 MB out store at ~400 GB/s x 0.83 util ~ 50.5 us
    (measured no-matmul probe: ~51-54 us). 1 byte/elem I/O is the floor.
  - PE: all-bf16 2-pass GEMM = 131k cycles = 54.6 us; fp8e4m3 DoubleRow
    (K=256 in one pass, 2 MACs/cell/cycle) halves whatever fraction runs it.
  - PSUM drains (fp32 PSUM -> fp8 SBUF): DVE is 1x-capped on fp32-PSUM reads
    ((120+512)/0.96 = 658 ns) vs ACT (172+512)/1.2 = 570 ns; the 7:8
    vec:scalar split balances both at ~39 us (2:1 put DVE at ~56 us and was
    the hidden co-bottleneck of the old all-f8h baseline).

- MODE "f8d" (default): hybrid DoubleRow. Host synthesizes per-sample W in
  fp64 and uploads both bf16 and e4m3(x16) forms; x ships as 1 byte/elem with
  per-512-col-tile encoding: DRF=14 of 32 j-tiles per sample (7 of every 16,
  interleaved so per-slice PE load stays uniform) are e4m3 and run the full
  K=256 contraction as ONE DoubleRow matmul (both fp8 operands required by
  HW), the rest are e3m4 and run the 2-pass bf16(W) path. e4m3 has 3 mantissa
  bits vs e3m4's 4, so the DR fraction is error-capped: sim/HW rel err
  1.22e-2 at DRF=0, 1.92e-2 at DRF=14 (HW-verified, gate 2e-2), 2.55e-2 at
  DRF=32. PE time drops 54.6 -> 42.7 us, landing at the DMA floor. Measured
  end-to-end vs the old all-f8h baseline in the same run: ~73 -> ~53 us
  (drain rebalance ~10%, DoubleRow + pipeline tuning the rest).
- MODE "f8h":   x e3m4, W bf16 (host-synthesized), out e3m4 + host residual.
- MODE "f8x":   like f8h but routing + weight synthesis run on device.
- MODE "f8o":   x/W bf16, float8_e3m4 out + host residual.
- MODE "f8io":  all-float8_e3m4, W prescaled x16 out of e3m4 denormal range.
- MODE "bf16":  x/W/out all bf16 (+I folded into W on host; host upcasts).
- MODE "f32r3": original compensated fp32 path (Wr@xr+Wr@xl+Wl@xr), exact to
                ~1e-7 but 2x the HBM traffic and 3x the PE work.
"""

import sys
from contextlib import ExitStack

import numpy as np
import ml_dtypes

_TRN_REPO = "/opt/trn_rl_repo"
if _TRN_REPO not in sys.path:
    sys.path.insert(0, _TRN_REPO)

B, C, H, W = 16, 256, 128, 128
E, D, HID = 3, 3, 16
HWP = H * W            # 16384 spatial positions
NCORES = 8
BLOC = B // NCORES     # 2 samples per core
P = 128                # partitions
KCH = C // P           # 2 row/contraction chunks
MMW = 512              # matmul free dim (one PSUM bank, fp32)

MODE = "f8d"           # "bf16" | "f8o" | "f8x" | "f8h" | "f8io" | "f32r3" | "f8d"
NW = 8192              # spatial slice width per DMA tile (f8h modes)
PROBE = "none"         # "none" | "pehalf" (bench-only) | "nomm" (bench-only)
STOREQ = "gpsimd"      # engine queue for output stores
DRAIN = 4              # PSUM drains: 1 = vector only, 2 = alternate, 3 = 2:1
                       # vec:scalar, 4 = 7:8 vec:scalar (cost-balanced: DVE
                       # fp32-PSUM reads are 1x-capped at (120+512)/0.96 =
                       # 658ns vs ACT (172+512)/1.2 = 570ns)
PSB = 1                # PSUM banks per tile (1 or 2); drains cover PSB*MMW
KORD = 0               # 1 = k-outer matmul order (4 j-tiles per weight load)
XBUFS = 40             # xin/oout pool depth (deeper DMA run-ahead)
PERF = "none"          # "none" | "dpix" (MatmulPerfMode.DoublePixel on main GEMM)

# ---- f8d (hybrid DoubleRow) knobs ----
# Per sample, DRF of the 32 512-col j-tiles (placed per DRP) run the CxC GEMM
# as ONE fp8e4m3 DoubleRow matmul (K=256 in a single PE pass, 2
# MACs/cell/cycle); the rest run the 2-pass bf16(W) x e3m4(x) path. e4m3 has
# 3 mantissa bits vs e3m4's 4, so the DR fraction is capped by the 2e-2
# rel-err gate: sim rel err is 1.22e-2 at DRF=0 and 2.55e-2 at DRF=32;
# DRF=12 lands at 1.835e-2 and DRF=14 at 1.919e-2 (both HW-verified exactly).
# x arrives as one byte/elem with per-j-tile encoding (e4m3 on DR tiles, e3m4
# elsewhere).
NWD = 1024             # f8d slice width (NSLD = HWP // NWD slices/sample);
                       # finer slices measured strictly faster (1024 < 2048 <
                       # 4096 < 8192): stores start earlier per slice and the
                       # load/store streams interleave more finely on the DMA
                       # fabric, tightening overlap toward the HBM floor
DRF = 14               # DoubleRow j-tiles per sample (of HWP//MMW = 32)
DMAQ = "base"          # "base" (loads sync, stores gpsimd) | "spread"
                       # (alternate queues per slice) | "split2" (half-DMAs on
                       # two queues each) — hides per-DMA DGE init latency
DRP = "mod16"          # DR tile placement: "block" (first DRF of 32) |
                       # "mod8"/"mod16" (DRF*8/32 of every 8 / DRF*16/32 of
                       # every 16 — spreads PE load evenly across slices so no
                       # slice leaves a >3.4us PE hole that would trip the HAM
                       # clock-gate re-throttle)

_CACHE = {}


def _is_dr(j):
    """Whether global j-tile (0..31, 512 cols each) runs the DoubleRow path."""
    if DRP == "mod8":
        assert DRF % 4 == 0
        return j % 8 < DRF // 4
    if DRP == "mod16":
        assert DRF % 2 == 0
        return j % 16 < DRF // 2
    return j < DRF

# dtype plumbing per mode:
# (x dtype, w dtype, out dtype, fold +I, host adds x, weight prescale)
# weight prescale: W is synthesized as wscale*W on device (folded into the
# one-hot selector rows on host) and the PSUM->SBUF drain divides it back out.
# This keeps the ~N(0, 0.09) weight entries out of float8e3's denormal range
# (min normal 0.25).
_MODES = {
    "bf16": ("bfloat16", "bfloat16", "bfloat16", True, False, 1.0),
    "f8o": ("bfloat16", "bfloat16", "float8e3", False, True, 1.0),
    "f8x": ("float8e3", "bfloat16", "float8e3", False, True, 1.0),
    "f8h": ("float8e3", "bfloat16", "float8e3", False, True, 1.0),
    "f8hw": ("float8e3", "float8e3", "float8e3", False, True, 16.0),
    "f8io": ("float8e3", "float8e3", "float8e3", False, True, 16.0),
    "f32r3": ("float32", "float32", "float32", True, False, 1.0),
}


def _pm(mybir):
    return mybir.MatmulPerfMode.DoublePixel if PERF == "dpix" else None


def _np_dt(name):
    return {
        "bfloat16": ml_dtypes.bfloat16,
        "float8e3": ml_dtypes.float8_e3m4,
        "float32": np.float32,
    }[name]


def _body(tc, bass, mybir, x_d, ew_d, st_d, f1w_d, f1b_d, f2w_d, f2b_d, sel_d, out_d):
    f32 = mybir.dt.float32
    f32r = mybir.dt.float32r
    AF = mybir.ActivationFunctionType
    AX = mybir.AxisListType
    xdt_n, wdt_n, odt_n, _fold, _hres, wscale = _MODES[MODE]
    xdt = getattr(mybir.dt, xdt_n)
    wdt = getattr(mybir.dt, wdt_n)
    odt = getattr(mybir.dt, odt_n)
    nc = tc.nc
    NSL = HWP // NW
    NSUB = NW // MMW
    with ExitStack() as ctx:
        cbuf = 1 if MODE == "f32r3" else 2
        const = ctx.enter_context(tc.tile_pool(name="const", bufs=cbuf))
        nbuf = XBUFS if XBUFS else (3 if MODE == "f32r3" else 4)
        xpool = ctx.enter_context(tc.tile_pool(name="xin", bufs=nbuf))
        opool = ctx.enter_context(tc.tile_pool(name="oout", bufs=nbuf))
        psum = ctx.enter_context(
            tc.tile_pool(name="psum", bufs=8 // PSB, space="PSUM")
        )
        if MODE == "f32r3":
            xrpool = ctx.enter_context(tc.tile_pool(name="xr", bufs=4))
            xlpool = ctx.enter_context(tc.tile_pool(name="xl", bufs=4))

        if MODE in ("f8h", "f8hw"):
            # routing + weight synthesis were done on host (fp64); load the
            # per-sample transposed weights wd[b][p, k*C+o] = W_b[o, k*128+p]
            wr_t = []
            for b in range(BLOC):
                wr = const.tile([P, KCH * C], wdt, name=f"wr{b}", tag=f"wr{b}")
                nc.sync.dma_start(
                    wr[:].rearrange("p (k o) -> p k o", k=KCH),
                    ew_d.ap()[b].rearrange("(k p) o -> p k o", p=P),
                )
                wr_t.append(wr)
            wl_t = []
        else:
            # ---- load constants ----
            # expert weights, transposed (+I folded on host when _fold):
            # ew_t[e][p, k*C+o] = expert_w[o, k*128+p] (+I)
            ew_t = []
            for e in range(E):
                t = const.tile([P, KCH * C], f32, name=f"ew{e}", tag=f"ew{e}")
                nc.sync.dma_start(
                    t[:].rearrange("p (k o) -> p k o", k=KCH),
                    ew_d.ap()[e].rearrange("(k p) o -> p k o", p=P),
                )
                ew_t.append(t)

            st_t = const.tile([D, BLOC], f32, name="st", tag="st")
            nc.sync.dma_start(st_t[:], st_d.ap())
            f1w_t = const.tile([D, HID], f32, name="f1w", tag="f1w")
            nc.sync.dma_start(f1w_t[:], f1w_d.ap())
            f1b_t = const.tile([HID, 1], f32, name="f1b", tag="f1b")
            nc.sync.dma_start(f1b_t[:], f1b_d.ap())
            f2w_t = const.tile([HID, E], f32, name="f2w", tag="f2w")
            nc.sync.dma_start(f2w_t[:], f2w_d.ap())
            f2b_t = const.tile([BLOC, E], f32, name="f2b", tag="f2b")
            nc.sync.dma_start(f2b_t[:], f2b_d.ap())

            # per-local-sample one-hot selector rows for the broadcast matmul
            sel_t = []
            for b in range(BLOC):
                s = const.tile([BLOC, P], f32, name=f"sel{b}", tag=f"sel{b}")
                nc.sync.dma_start(s[:], sel_d.ap()[b])
                sel_t.append(s)

            # ---- routing MLP (B on the free axis, all samples of this core) ----
            h_ps = psum.tile([HID, BLOC], f32, name="h_ps", tag="mm")
            nc.tensor.matmul(h_ps[:], f1w_t[:], st_t[:])
            hT = const.tile([HID, BLOC], f32, name="hT", tag="hT")
            nc.scalar.activation(hT[:], h_ps[:], AF.Relu, bias=f1b_t[:, 0:1], scale=1.0)

            lg_ps = psum.tile([BLOC, E], f32, name="lg_ps", tag="mm")
            nc.tensor.matmul(lg_ps[:], hT[:], f2w_t[:])
            lg = const.tile([BLOC, E], f32, name="lg", tag="lg")
            nc.vector.tensor_add(lg[:], lg_ps[:], f2b_t[:])

            # softmax along free axis (E=3)
            mx = const.tile([BLOC, 1], f32, name="mx", tag="mx")
            nc.vector.reduce_max(mx[:], lg[:], axis=AX.X)
            nmx = const.tile([BLOC, 1], f32, name="nmx", tag="nmx")
            nc.vector.tensor_scalar_mul(nmx[:], mx[:], -1.0)
            exps = const.tile([BLOC, E], f32, name="exps", tag="exps")
            nc.scalar.activation(exps[:], lg[:], AF.Exp, bias=nmx[:, 0:1], scale=1.0)
            sm = const.tile([BLOC, 1], f32, name="sm", tag="sm")
            nc.vector.reduce_sum(sm[:], exps[:], axis=AX.X)
            rcp = const.tile([BLOC, 1], f32, name="rcp", tag="rcp")
            nc.vector.reciprocal(rcp[:], sm[:])
            r_t = const.tile([BLOC, E], f32, name="r_t", tag="r_t")
            nc.vector.tensor_scalar_mul(r_t[:], exps[:], rcp[:, 0:1])

            # ---- per-sample dynamic weight synthesis ----
            wb_t, wr_t, wl_t = [], [], []
            for b in range(BLOC):
                rb_ps = psum.tile([P, E], f32, name=f"rb_ps{b}", tag="mm")
                nc.tensor.matmul(rb_ps[:], sel_t[b][:], r_t[:])
                rb = const.tile([P, E], f32, name=f"rb{b}", tag=f"rb{b}")
                nc.vector.tensor_copy(rb[:], rb_ps[:])

                # wb is dead once the cast copy is derived, so both samples share
                # one slot except in pure-fp32 mode
                wb_tag = f"wb{b}" if MODE == "f32r3" else "wb"
                wb = const.tile([P, KCH * C], f32, name=f"wb{b}", tag=wb_tag)
                tmp = const.tile([P, KCH * C], f32, name=f"wtmp{b}", tag="wtmp")
                nc.vector.tensor_scalar_mul(wb[:], ew_t[0][:], rb[:, 0:1])
                nc.vector.tensor_scalar_mul(tmp[:], ew_t[1][:], rb[:, 1:2])
                nc.vector.tensor_add(wb[:], wb[:], tmp[:])
                nc.vector.tensor_scalar_mul(tmp[:], ew_t[2][:], rb[:, 2:3])
                nc.vector.tensor_add(wb[:], wb[:], tmp[:])
                wb_t.append(wb)

                if MODE == "f32r3":
                    wr = const.tile([P, KCH * C], f32r, name=f"wr{b}", tag=f"wr{b}")
                    nc.vector.tensor_copy(wr[:], wb[:])
                    wr_t.append(wr)
                    wl = const.tile([P, KCH * C], f32r, name=f"wl{b}", tag=f"wl{b}")
                    nc.vector.tensor_sub(wl[:], wb[:], wr[:].bitcast(f32))
                    wl_t.append(wl)
                else:
                    wr = const.tile([P, KCH * C], wdt, name=f"wr{b}", tag=f"wr{b}")
                    nc.vector.tensor_copy(wr[:], wb[:])
                    wr_t.append(wr)

        # ---- main GEMM: out[b, o, n] = sum_c w'[o, c] x[b, c, n] ----
        # One merged 3D-AP DMA per slice on each side: the load covers both
        # k-chunks ([p, k, n]), the store covers both m-chunks ([p, m, n]).
        for b in range(BLOC):
            x_b = x_d.ap()[b].rearrange("(k p) n -> p k n", p=P)
            o_b = out_d.ap()[b].rearrange("(m p) n -> p m n", p=P)
            for s in range(NSL):
                ns = slice(s * NW, (s + 1) * NW)
                xt = xpool.tile([P, KCH * NW], xdt, name=f"x{b}_{s}", tag="x")
                if b == 0 and s == 0:
                    # split the very first load per k-chunk so the first
                    # matmuls start ~a DMA earlier
                    for k in range(KCH):
                        nc.sync.dma_start(
                            xt[:, k * NW : (k + 1) * NW], x_b[:, k, ns]
                        )
                else:
                    nc.sync.dma_start(
                        xt[:].rearrange("p (k n) -> p k n", k=KCH), x_b[:, :, ns]
                    )
                xk = [xt[:, k * NW : (k + 1) * NW] for k in range(KCH)]
                xrk, xlk = [], []
                if MODE == "f32r3":
                    for k in range(KCH):
                        xr = xrpool.tile([P, NW], f32r, name=f"xr{b}_{s}_{k}", tag="xr")
                        nc.scalar.copy(xr[:], xk[k])
                        xrk.append(xr)
                        xl = xlpool.tile([P, NW], f32r, name=f"xl{b}_{s}_{k}", tag="xl")
                        nc.vector.tensor_sub(xl[:], xk[k], xr[:].bitcast(f32))
                        xlk.append(xl)
                ot = opool.tile([P, KCH * NW], odt, name=f"o{b}_{s}", tag="o")
                if KORD and MODE not in ("f32r3",) and PSB == 1 and wscale == 1.0:
                    GRP = 4
                    for m in range(KCH):
                        for g in range(NSUB // GRP):
                            pss = [
                                psum.tile(
                                    [P, MMW], f32,
                                    name=f"mm{b}_{s}_{m}_{g}_{jj}", tag="mm",
                                )
                                for jj in range(GRP)
                            ]
                            for k in range(KCH):
                                wsl = wr_t[b][:, k * C + m * P : k * C + m * P + P]
                                for jj in range(GRP):
                                    j = g * GRP + jj
                                    rs = slice(j * MMW, (j + 1) * MMW)
                                    nc.tensor.matmul(
                                        pss[jj][:], wsl, xk[k][:, rs],
                                        start=(k == 0), stop=(k == KCH - 1),
                                        perf_mode=_pm(mybir),
                                    )
                            for jj in range(GRP):
                                j = g * GRP + jj
                                js = slice(m * NW + j * MMW, m * NW + (j + 1) * MMW)
                                if jj % 2 == 0:
                                    nc.vector.tensor_copy(ot[:, js], pss[jj][:])
                                else:
                                    nc.scalar.copy(ot[:, js], pss[jj][:])
                    if b == BLOC - 1 and s == NSL - 1:
                        for m in range(KCH):
                            getattr(nc, STOREQ).dma_start(
                                o_b[:, m, ns], ot[:, m * NW : (m + 1) * NW]
                            )
                    else:
                        getattr(nc, STOREQ).dma_start(
                            o_b[:, :, ns], ot[:].rearrange("p (m n) -> p m n", m=KCH)
                        )
                    continue
                for m in range(KCH):
                    for j2 in range(NSUB // PSB):
                        ps = psum.tile(
                            [P, PSB * MMW], f32, name=f"mm{b}_{s}_{m}_{j2}", tag="mm"
                        )
                        for q in range(PSB):
                            j = j2 * PSB + q
                            rs = slice(j * MMW, (j + 1) * MMW)
                            qs = slice(q * MMW, (q + 1) * MMW)
                            if MODE == "f32r3":
                                mms = []
                                for k in range(KCH):
                                    mms.append((wr_t[b], xrk[k][:, rs], k))
                                    mms.append((wr_t[b], xlk[k][:, rs], k))
                                    mms.append((wl_t[b], xrk[k][:, rs], k))
                            else:
                                mms = [(wr_t[b], xk[k][:, rs], k) for k in range(KCH)]
                            if PROBE == "pehalf":
                                mms = mms[:1]
                            elif PROBE == "nomm":
                                mms = []
                            for i, (wt, rhs, k) in enumerate(mms):
                                nc.tensor.matmul(
                                    ps[:, qs],
                                    wt[:, k * C + m * P : k * C + m * P + P],
                                    rhs,
                                    start=(i == 0),
                                    stop=(i == len(mms) - 1),
                                    perf_mode=_pm(mybir),
                                )
                        js = slice(
                            m * NW + j2 * PSB * MMW, m * NW + (j2 + 1) * PSB * MMW
                        )
                        inv = None if wscale == 1.0 else 1.0 / wscale
                        idx = m * (NSUB // PSB) + j2
                        if DRAIN == 3:
                            di = 0 if idx % 3 < 2 else 1  # 2:1 vector:scalar
                        elif DRAIN == 4:
                            di = 1 if idx % 15 % 2 == 0 else 0  # 8 sc : 7 vec
                        else:
                            di = idx % DRAIN
                        if inv is None:
                            if di == 0:
                                nc.vector.tensor_copy(ot[:, js], ps[:])
                            else:
                                nc.scalar.copy(ot[:, js], ps[:])
                        else:
                            if di == 0:
                                nc.vector.tensor_scalar_mul(ot[:, js], ps[:], inv)
                            else:
                                nc.scalar.activation(
                                    ot[:, js], ps[:], AF.Copy, scale=inv
                                )
                if b == BLOC - 1 and s == NSL - 1:
                    # split the very last store per m-chunk so the pipeline
                    # tail drains with a smaller final DMA
                    for m in range(KCH):
                        getattr(nc, STOREQ).dma_start(
                            o_b[:, m, ns], ot[:, m * NW : (m + 1) * NW]
                        )
                else:
                    getattr(nc, STOREQ).dma_start(
                        o_b[:, :, ns], ot[:].rearrange("p (m n) -> p m n", m=KCH)
                    )


def _body_f8d(tc, bass, mybir, x_d, ew_d, ew4_d, out_d):
    """Hybrid DoubleRow body: per sample, the first DRF of the 32 512-col
    j-tiles run the full K=256 contraction as one fp8e4m3 DoubleRow matmul;
    the rest run the 2-pass bf16(W) x e3m4(x) path. x arrives as one
    byte/elem with per-j-tile encoding (e4m3 on DR tiles, e3m4 elsewhere);
    the dram tensor dtype is float8e4 and non-DR tiles bitcast to float8e3."""
    f32 = mybir.dt.float32
    AF = mybir.ActivationFunctionType
    e4 = mybir.dt.float8e4
    e3 = mybir.dt.float8e3
    bf16 = mybir.dt.bfloat16
    odt = e3
    DR = mybir.MatmulPerfMode.DoubleRow
    nc = tc.nc
    NSLD = HWP // NWD
    NSUB = NWD // MMW
    assert DRF % PSB == 0 and NSUB % PSB == 0
    assert all(
        _is_dr(g * PSB) == _is_dr(g * PSB + q)
        for g in range(HWP // MMW // PSB)
        for q in range(PSB)
    ), "PSB group straddles the DR/normal boundary"
    with ExitStack() as ctx:
        const = ctx.enter_context(tc.tile_pool(name="const", bufs=1))
        xpool = ctx.enter_context(tc.tile_pool(name="xin", bufs=XBUFS))
        opool = ctx.enter_context(tc.tile_pool(name="oout", bufs=XBUFS))
        psum = ctx.enter_context(
            tc.tile_pool(name="psum", bufs=8 // PSB, space="PSUM")
        )

        # per-sample weights: bf16 (normal path) + e4m3 x16 (DoubleRow path),
        # both transposed: w[p, k*C+o] = W_b[o, k*128+p]
        wr_t, wdr_t = [], []
        for b in range(BLOC):
            wr = const.tile([P, KCH * C], bf16, name=f"wr{b}", tag=f"wr{b}")
            nc.sync.dma_start(
                wr[:].rearrange("p (k o) -> p k o", k=KCH),
                ew_d.ap()[b].rearrange("(k p) o -> p k o", p=P),
            )
            wr_t.append(wr)
            wd = const.tile([P, KCH * C], e4, name=f"wdr{b}", tag=f"wdr{b}")
            nc.sync.dma_start(
                wd[:].rearrange("p (k o) -> p k o", k=KCH),
                ew4_d.ap()[b].rearrange("(k p) o -> p k o", p=P),
            )
            wdr_t.append(wd)

        # bench-only probe: dummy psums written once; drains read them instead
        # of live matmul output, making the kernel a pure DMA+drain pipeline
        # (rotating dummies so reads spread over PSUM banks as in real runs)
        dummies = []
        if PROBE == "nomm":
            for d in range(8 // PSB):
                dm = psum.tile([P, PSB * MMW], f32, name=f"dummy{d}", tag="mm")
                for q in range(PSB):
                    nc.tensor.matmul(
                        dm[:, q * MMW : (q + 1) * MMW], wdr_t[0][:, 0:P],
                        wdr_t[0][:, 0:MMW], start=True, stop=True,
                    )
                dummies.append(dm)

        didx = 0  # running drain index for the 2:1 vec:scalar split

        def drain(dst, src, scale):
            nonlocal didx
            if DRAIN == 3:
                di = 0 if didx % 3 < 2 else 1
            elif DRAIN == 4:
                di = 1 if didx % 15 % 2 == 0 else 0  # 8 scalar : 7 vector
            else:
                di = didx % max(DRAIN, 1)
            didx += 1
            if scale is None:
                if di == 0:
                    nc.vector.tensor_copy(dst, src)
                else:
                    nc.scalar.copy(dst, src)
            else:
                if di == 0:
                    nc.vector.tensor_scalar_mul(dst, src, scale)
                else:
                    nc.scalar.activation(dst, src, AF.Copy, scale=scale)

        for b in range(BLOC):
            x_b = x_d.ap()[b].rearrange("(k p) n -> p k n", p=P)
            o_b = out_d.ap()[b].rearrange("(m p) n -> p m n", p=P)
            for s in range(NSLD):
                si = b * NSLD + s
                ns = slice(s * NWD, (s + 1) * NWD)
                xt = xpool.tile([P, KCH * NWD], e4, name=f"x{b}_{s}", tag="x")
                src = x_b[:, :, ns]
                if DMAQ == "spread":
                    ldq = [nc.sync, nc.scalar][si % 2]
                elif DMAQ == "lds":
                    # alternate loads between sync and gpsimd so per-DMA DGE
                    # init latency overlaps across queues (gpsimd also issues
                    # stores but its SWDGE budget has headroom; scalar/vector
                    # are drain-busy and off limits)
                    ldq = [nc.sync, nc.gpsimd][si % 2]
                else:
                    ldq = nc.sync
                if DMAQ == "split2" or (b == 0 and s == 0):
                    # per-k-chunk half loads: the very first so the first
                    # matmuls start ~a DMA earlier; all of them in split2 so
                    # two queues' DGE inits overlap
                    for k in range(KCH):
                        q = [nc.sync, nc.scalar][k] if DMAQ == "split2" else ldq
                        q.dma_start(xt[:, k * NWD : (k + 1) * NWD], src[:, k])
                else:
                    ldq.dma_start(
                        xt[:].rearrange("p (k n) -> p k n", k=KCH), src
                    )
                ot = opool.tile([P, KCH * NWD], odt, name=f"o{b}_{s}", tag="o")
                xr3 = xt[:].rearrange("p (k n) -> p k n", k=KCH)
                wv = wdr_t[b][:].rearrange("p (k o) -> p k o", k=KCH)
                for m in range(KCH):
                    wsl = wv[:, :, m * P : (m + 1) * P]
                    for j2 in range(NSUB // PSB):
                        dr = _is_dr(s * NSUB + j2 * PSB)
                        js = slice(
                            m * NWD + j2 * PSB * MMW,
                            m * NWD + (j2 + 1) * PSB * MMW,
                        )
                        scale = 1.0 / 16.0 if dr else None
                        if PROBE == "nomm":
                            drain(ot[:, js], dummies[didx % (8 // PSB)][:], scale)
                            continue
                        ps = psum.tile(
                            [P, PSB * MMW], f32, name=f"mm{b}_{s}_{m}_{j2}",
                            tag="mm",
                        )
                        for q in range(PSB):
                            j = j2 * PSB + q
                            rs = slice(j * MMW, (j + 1) * MMW)
                            qs = slice(q * MMW, (q + 1) * MMW)
                            if dr:
                                nc.tensor.matmul(
                                    ps[:, qs], wsl, xr3[:, :, rs], start=True,
                                    stop=True, perf_mode=DR,
                                )
                            else:
                                nk = 1 if PROBE == "pehalf" else KCH
                                for k in range(nk):
                                    nc.tensor.matmul(
                                        ps[:, qs],
                                        wr_t[b][:, k * C + m * P : k * C + m * P + P],
                                        xt[:, k * NWD + j * MMW : k * NWD + (j + 1) * MMW].bitcast(e3),
                                        start=(k == 0),
                                        stop=(k == nk - 1),
                                    )
                        drain(ot[:, js], ps[:], scale)
                stq = getattr(nc, STOREQ)
                if DMAQ in ("split2", "msplit") or (
                    b == BLOC - 1 and s == NSLD - 1
                ):
                    # per-m-chunk half stores: the very last so the pipeline
                    # tail drains with a smaller final DMA; all in split2
                    # (gpsimd + sync so two queues' DGE inits overlap) and in
                    # msplit (both on STOREQ — subtile deps let the m=0 half
                    # store as soon as its drains land, half a slice early)
                    for m in range(KCH):
                        q = (
                            [getattr(nc, STOREQ), nc.sync][m]
                            if DMAQ == "split2" else stq
                        )
                        q.dma_start(
                            o_b[:, m, ns], ot[:, m * NWD : (m + 1) * NWD]
                        )
                else:
                    stq.dma_start(
                        o_b[:, :, ns], ot[:].rearrange("p (m n) -> p m n", m=KCH)
                    )


def _build(reps=1, barrier=False):
    import concourse.bacc as bacc
    import concourse.bass as bass
    import concourse.tile as tile
    from concourse import mybir

    f32 = mybir.dt.float32
    if MODE == "f8d":
        nc = bacc.Bacc(
            "TRN2", target_bir_lowering=False, debug=False, num_devices=NCORES
        )
        x_d = nc.dram_tensor(
            "x", [BLOC, C, HWP], mybir.dt.float8e4, kind="ExternalInput"
        )
        ew_d = nc.dram_tensor(
            "ew", [BLOC, C, C], mybir.dt.bfloat16, kind="ExternalInput"
        )
        ew4_d = nc.dram_tensor(
            "ew4", [BLOC, C, C], mybir.dt.float8e4, kind="ExternalInput"
        )
        out_d = nc.dram_tensor(
            "out", [BLOC, C, HWP], mybir.dt.float8e3, kind="ExternalOutput"
        )
        with tile.TileContext(nc) as tc:
            for i in range(reps):
                _body_f8d(tc, bass, mybir, x_d, ew_d, ew4_d, out_d)
                if barrier and i < reps - 1:
                    tc.strict_bb_all_engine_barrier()
        nc.compile()
        return nc
    xdt_n, wdt_n, odt_n, _fold, _hres, wscale = _MODES[MODE]
    xdt = getattr(mybir.dt, xdt_n)
    odt = getattr(mybir.dt, odt_n)
    wdt = getattr(mybir.dt, wdt_n)
    nc = bacc.Bacc("TRN2", target_bir_lowering=False, debug=False, num_devices=NCORES)
    x_d = nc.dram_tensor("x", [BLOC, C, HWP], xdt, kind="ExternalInput")
    if MODE in ("f8h", "f8hw"):
        ew_d = nc.dram_tensor("ew", [BLOC, C, C], wdt, kind="ExternalInput")
    else:
        ew_d = nc.dram_tensor("ew", [E, C, C], f32, kind="ExternalInput")
    st_d = nc.dram_tensor("scoresT", [D, BLOC], f32, kind="ExternalInput")
    f1w_d = nc.dram_tensor("fc1_w", [D, HID], f32, kind="ExternalInput")
    f1b_d = nc.dram_tensor("fc1_b", [HID, 1], f32, kind="ExternalInput")
    f2w_d = nc.dram_tensor("fc2_w", [HID, E], f32, kind="ExternalInput")
    f2b_d = nc.dram_tensor("fc2_b_rep", [BLOC, E], f32, kind="ExternalInput")
    sel_d = nc.dram_tensor("sel", [BLOC, BLOC, P], f32, kind="ExternalInput")
    out_d = nc.dram_tensor("out", [BLOC, C, HWP], odt, kind="ExternalOutput")
    with tile.TileContext(nc) as tc:
        for i in range(reps):
            _body(
                tc, bass, mybir, x_d, ew_d, st_d, f1w_d, f1b_d, f2w_d, f2b_d, sel_d,
                out_d,
            )
            if barrier and i < reps - 1:
                tc.strict_bb_all_engine_barrier()
    nc.compile()
    return nc


def _get_nc(reps=1, barrier=False):
    key = (
        "nc", MODE, NW, PROBE, STOREQ, DRAIN, PSB, KORD, XBUFS, PERF, NWD, DRF,
        DMAQ, DRP, reps, barrier,
    )
    if key not in _CACHE:
        _CACHE[key] = _build(reps, barrier)
    return _CACHE[key]


def _routing_dyn_w(inputs):
    """fp64 routing MLP + weight synthesis on host -> dyn W [B, O, C]."""
    scores = np.asarray(inputs["scores"], dtype=np.float64)
    fc1_w = np.asarray(inputs["fc1_w"], dtype=np.float64)
    fc1_b = np.asarray(inputs["fc1_b"], dtype=np.float64)
    fc2_w = np.asarray(inputs["fc2_w"], dtype=np.float64)
    fc2_b = np.asarray(inputs["fc2_b"], dtype=np.float64)
    expert_w = np.asarray(inputs["expert_w"], dtype=np.float64)
    h = np.maximum(scores @ fc1_w + fc1_b, 0.0)
    z = h @ fc2_w + fc2_b
    r = np.exp(z - z.max(1, keepdims=True))
    r /= r.sum(1, keepdims=True)
    return np.einsum("be,eoc->boc", r, expert_w)


def make_in_maps(inputs):
    """Shard FULL inputs into 8 per-core input maps (host-side layout prep only)."""
    if MODE == "f8d":
        e4 = ml_dtypes.float8_e4m3
        e3 = ml_dtypes.float8_e3m4
        dyn_w = _routing_dyn_w(inputs)
        ewT = dyn_w.transpose(0, 2, 1)  # [B, c_in, c_out]
        ew_bf = np.ascontiguousarray(ewT.astype(ml_dtypes.bfloat16))
        ew4 = np.ascontiguousarray((ewT * 16.0).astype(np.float32).astype(e4))
        x = np.asarray(inputs["x"], dtype=np.float32).reshape(B, C, HWP)
        xv = x.reshape(B, C, HWP // MMW, MMW)
        xb = np.empty((B, C, HWP // MMW, MMW), dtype=np.uint8)
        drj = np.array([_is_dr(j) for j in range(HWP // MMW)])
        xb[:, :, drj] = xv[:, :, drj].astype(e4).view(np.uint8)
        xb[:, :, ~drj] = xv[:, :, ~drj].astype(e3).view(np.uint8)
        xb = xb.reshape(B, C, HWP).view(e4)
        in_maps = []
        for c in range(NCORES):
            g0 = c * BLOC
            in_maps.append(
                {
                    "x": xb[g0 : g0 + BLOC],
                    "ew": ew_bf[g0 : g0 + BLOC],
                    "ew4": ew4[g0 : g0 + BLOC],
                }
            )
        return in_maps
    xdt_n, _wdt_n, _odt_n, fold, _hres, wscale = _MODES[MODE]
    x = np.ascontiguousarray(np.asarray(inputs["x"], dtype=np.float32))
    scores = np.asarray(inputs["scores"], dtype=np.float32)
    fc1_w = np.ascontiguousarray(np.asarray(inputs["fc1_w"], dtype=np.float32))
    fc1_b = np.asarray(inputs["fc1_b"], dtype=np.float32)
    fc2_w = np.ascontiguousarray(np.asarray(inputs["fc2_w"], dtype=np.float32))
    fc2_b = np.asarray(inputs["fc2_b"], dtype=np.float32)
    expert_w = np.asarray(inputs["expert_w"], dtype=np.float32)

    if MODE in ("f8h", "f8hw"):
        # routing MLP + weight synthesis in fp64 on host
        h = np.maximum(scores.astype(np.float64) @ fc1_w.astype(np.float64)
                       + fc1_b.astype(np.float64), 0.0)
        z = h @ fc2_w.astype(np.float64) + fc2_b.astype(np.float64)
        r = np.exp(z - z.max(1, keepdims=True))
        r /= r.sum(1, keepdims=True)
        dyn_w = np.einsum("be,eoc->boc", r, expert_w.astype(np.float64))
        # transpose to [b, c_in, c_out], prescale (see wscale note above) and
        # cast to the matmul weight dtype
        ew = np.ascontiguousarray(
            (dyn_w.transpose(0, 2, 1) * wscale).astype(_np_dt(_wdt_n))
        )
    else:
        # transpose experts to [e, c_in, c_out]; fold the residual identity
        # when the device computes (I+W) @ x directly
        ew = np.ascontiguousarray(expert_w.transpose(0, 2, 1))
        if fold:
            idx = np.arange(C)
            ew[:, idx, idx] += np.float32(1.0)

    x_r = np.ascontiguousarray(x.reshape(B, C, HWP).astype(_np_dt(xdt_n)))
    f1b = np.ascontiguousarray(fc1_b.reshape(HID, 1))
    f2b = np.ascontiguousarray(np.tile(fc2_b.reshape(1, E), (BLOC, 1)))
    sel = np.zeros((BLOC, BLOC, P), dtype=np.float32)
    for b in range(BLOC):
        sel[b, b, :] = np.float32(wscale)

    in_maps = []
    for c in range(NCORES):
        g0 = c * BLOC
        in_maps.append(
            {
                "x": x_r[g0 : g0 + BLOC],
                "ew": ew[g0 : g0 + BLOC] if MODE in ("f8h", "f8hw") else ew,
                "scoresT": np.ascontiguousarray(scores[g0 : g0 + BLOC].T),
                "fc1_w": fc1_w,
                "fc1_b": f1b,
                "fc2_w": fc2_w,
                "fc2_b_rep": f2b,
                "sel": sel,
            }
        )
    return in_maps


def run_spmd(inputs, trace=False):
    """Run the Bass kernel on cores 0-7. Returns BassKernelResults."""
    import os

    from concourse import bass_utils

    nc = _get_nc()
    in_maps = make_in_maps(inputs)
    try:
        return bass_utils.run_bass_kernel_spmd(
            nc, in_maps, core_ids=list(range(NCORES)), trace=trace
        )
    except ModuleNotFoundError as e:
        # BASS_TRACE set in an env without the axon NTFF hook module:
        # fall back to untraced execution instead of crashing
        if "antenv" not in str(e) and "axon" not in str(e):
            raise
        os.environ["BASS_NEVER_TRACE"] = "1"
        try:
            return bass_utils.run_bass_kernel_spmd(
                nc, in_maps, core_ids=list(range(NCORES)), trace=False
            )
        finally:
            os.environ.pop("BASS_NEVER_TRACE", None)


def kernel(**inputs) -> np.ndarray:
    if MODE == "f8d":
        hres = True
    else:
        _xdt_n, _wdt_n, _odt_n, _fold, hres, _wscale = _MODES[MODE]
    res = run_spmd(inputs, trace=False)
    out = np.stack([np.asarray(r["out"]) for r in res.results], axis=0)
    out = out.astype(np.float32).reshape(B, C, H, W)
    if hres:
        out += np.asarray(inputs["x"], dtype=np.float32)
    return out

